# revision 8
# baseline (speedup 1.0000x reference)
"""Trainium2 Bass kernel for the CoupledTauModel (gnn_message_passing).

Strategy
--------
All math runs in a transposed "nodes-on-partitions" layout: the state lives as
usT/ufT [N, B] sharded so that core c owns output rows [c*LOC, (c+1)*LOC).

Host-side prep folds the Euler update into a single streamed matrix per state:
    A_s = I - dt*c_s*Ls.T          (shape [N, N], core c gets columns c-slice)
so one PSUM accumulation computes  us + dt*(-c_s * us@Ls.T)  directly, and the
low-rank coupling term  dt*l_s * uf@Ms.T  is added into the same PSUM group via
two tiny matmuls (rank 5).  Epilogue is a single Relu activation per tile.

Per step each core:
  1. p_sT = Ms_B.T @ ufT, p_fT = Mf_B.T @ usT        (tiny, rank-5)
  2. for its 1024 output rows: psum = A_s_slice.T @ usT + MsA_scaled.T @ p_sT
  3. new local state = Relu(psum)
  4. AllGather the [2*LOC, B] local state update across the 8 cores.

The 256MB Ls/Lf-derived matrices are streamed from HBM every step (they cannot
fit in SBUF) in 512KB contiguous slabs -> the kernel is HBM-bandwidth-bound,
which is the target regime.

The gate MLP (sigmoid split of x0) and decoder MLP run on-device in the same
transposed layout; W1/D1 are replicated, W2/D2 column-sharded.
"""

import math
from contextlib import ExitStack

import numpy as np

import concourse.bass as bass
import concourse.mybir as mybir
import concourse.tile as tile
from concourse import bacc
from concourse.bass_utils import run_bass_kernel_spmd

AF = mybir.ActivationFunctionType
FP32 = mybir.dt.float32

# Full-size problem config (hardcoded; the harness always uses this shape).
N_FULL = 8192
B_FULL = 8
H_FULL = 128
R_FULL = 5
NCORES = 8
NSTEPS_FULL = 10
DT = 0.1


def build_program(N=N_FULL, B=B_FULL, H=H_FULL, R=R_FULL, ncores=NCORES,
                  nsteps=NSTEPS_FULL, slab_bufs=16):
    """Build the SPMD Bass program (identical on every core; per-core data
    comes from each core's input map)."""
    LOC = N // ncores          # output rows owned per core
    KT = N // 128              # contraction tiles
    MT = LOC // 128            # output tiles per core
    assert N % (128 * ncores) == 0 and H == 128

    nc = bacc.Bacc("TRN2", target_bir_lowering=False, debug=False,
                   num_devices=ncores)
    dt = FP32

    x0T = nc.dram_tensor("x0T", [N, B], dt, kind="ExternalInput")
    x0Tl = nc.dram_tensor("x0Tl", [LOC, B], dt, kind="ExternalInput")
    As_d = nc.dram_tensor("As", [N, LOC], dt, kind="ExternalInput")
    Af_d = nc.dram_tensor("Af", [N, LOC], dt, kind="ExternalInput")
    MsB_d = nc.dram_tensor("MsB", [N, R], dt, kind="ExternalInput")
    MfB_d = nc.dram_tensor("MfB", [N, R], dt, kind="ExternalInput")
    MsA_d = nc.dram_tensor("MsA", [R, LOC], dt, kind="ExternalInput")
    MfA_d = nc.dram_tensor("MfA", [R, LOC], dt, kind="ExternalInput")
    W1_d = nc.dram_tensor("W1", [N, H], dt, kind="ExternalInput")
    b1_d = nc.dram_tensor("b1", [H, 1], dt, kind="ExternalInput")
    W2_d = nc.dram_tensor("W2", [H, LOC], dt, kind="ExternalInput")
    b2T_d = nc.dram_tensor("b2T", [128, MT], dt, kind="ExternalInput")
    D1_d = nc.dram_tensor("D1", [N, H], dt, kind="ExternalInput")
    db1_d = nc.dram_tensor("db1", [H, 1], dt, kind="ExternalInput")
    D2_d = nc.dram_tensor("D2", [H, LOC], dt, kind="ExternalInput")
    db2T_d = nc.dram_tensor("db2T", [128, MT], dt, kind="ExternalInput")

    x1_o = nc.dram_tensor("x1_o", [LOC, B], dt, kind="ExternalOutput")
    us_o = nc.dram_tensor("us_o", [LOC, B], dt, kind="ExternalOutput")
    uf_o = nc.dram_tensor("uf_o", [LOC, B], dt, kind="ExternalOutput")

    RG = [list(range(ncores))]

    with ExitStack() as ctx:
        tc = ctx.enter_context(tile.TileContext(nc))
        const = ctx.enter_context(tc.tile_pool(name="const", bufs=1))
        statep = ctx.enter_context(tc.tile_pool(name="state", bufs=2))
        slabp = ctx.enter_context(tc.tile_pool(name="slab", bufs=slab_bufs))
        mlpp = ctx.enter_context(tc.tile_pool(name="mlp", bufs=4))
        smallp = ctx.enter_context(tc.tile_pool(name="small", bufs=4))
        newp = ctx.enter_context(tc.tile_pool(name="new", bufs=2))
        psum = ctx.enter_context(tc.tile_pool(name="psum", bufs=8, space="PSUM"))
        dram = ctx.enter_context(tc.tile_pool(name="dram", bufs=2, space="DRAM"))

        # ---- resident constants ----
        x0T_sb = const.tile([128, KT, B], dt, tag="x0T")
        nc.sync.dma_start(x0T_sb[:], x0T[:].rearrange("(k p) b -> p k b", p=128))
        x0Tl_sb = const.tile([128, MT, B], dt, tag="x0Tl")
        nc.sync.dma_start(x0Tl_sb[:], x0Tl[:].rearrange("(m p) b -> p m b", p=128))
        MsB_sb = const.tile([128, KT, R], dt, tag="MsB")
        nc.sync.dma_start(MsB_sb[:], MsB_d[:].rearrange("(k p) r -> p k r", p=128))
        MfB_sb = const.tile([128, KT, R], dt, tag="MfB")
        nc.sync.dma_start(MfB_sb[:], MfB_d[:].rearrange("(k p) r -> p k r", p=128))
        MsA_sb = const.tile([R, LOC], dt, tag="MsA")
        nc.sync.dma_start(MsA_sb[:], MsA_d[:])
        MfA_sb = const.tile([R, LOC], dt, tag="MfA")
        nc.sync.dma_start(MfA_sb[:], MfA_d[:])
        b1_sb = const.tile([H, 1], dt, tag="b1")
        nc.sync.dma_start(b1_sb[:], b1_d[:])
        db1_sb = const.tile([H, 1], dt, tag="db1")
        nc.sync.dma_start(db1_sb[:], db1_d[:])
        b2_sb = const.tile([128, MT], dt, tag="b2")
        nc.sync.dma_start(b2_sb[:], b2T_d[:])
        db2_sb = const.tile([128, MT], dt, tag="db2")
        nc.sync.dma_start(db2_sb[:], db2T_d[:])

        # ---- gate MLP:  gate = sigmoid(relu(x0@W1+b1)@W2+b2) ----
        hpsum = psum.tile([H, B], dt, tag="mm")
        for k in range(KT):
            w1t = mlpp.tile([128, H], dt, tag="w1")
            nc.sync.dma_start(w1t[:], W1_d[k * 128:(k + 1) * 128, :])
            nc.tensor.matmul(hpsum[:], w1t[:], x0T_sb[:, k, :],
                             start=(k == 0), stop=(k == KT - 1))
        hT = smallp.tile([H, B], dt, tag="hT")
        nc.scalar.activation(hT[:], hpsum[:], AF.Relu, bias=b1_sb[:, 0:1])

        w2_sb = mlpp.tile([H, LOC], dt, tag="w2")
        nc.sync.dma_start(w2_sb[:], W2_d[:])
        news0 = newp.tile([128, MT, B], dt, tag="new")
        newf0 = newp.tile([128, MT, B], dt, tag="newf")
        for m in range(MT):
            gpsum = psum.tile([128, B], dt, tag="mm")
            nc.tensor.matmul(gpsum[:], w2_sb[:, m * 128:(m + 1) * 128], hT[:],
                             start=True, stop=True)
            gate = smallp.tile([128, B], dt, tag="gate")
            nc.scalar.activation(gate[:], gpsum[:], AF.Sigmoid, bias=b2_sb[:, m:m + 1])
            nc.vector.tensor_mul(news0[:, m, :], gate[:], x0Tl_sb[:, m, :])
            nc.vector.tensor_sub(newf0[:, m, :], x0Tl_sb[:, m, :], news0[:, m, :])

        def exchange(news, newf):
            """AllGather local [2*LOC, B] update -> fresh full-state tiles."""
            agin = dram.tile([2 * LOC, B], dt, tag="agin")
            nc.sync.dma_start(
                agin[0:LOC, :].rearrange("(m p) b -> p m b", p=128), news[:])
            nc.sync.dma_start(
                agin[LOC:2 * LOC, :].rearrange("(m p) b -> p m b", p=128), newf[:])
            agout = dram.tile([ncores * 2 * LOC, B], dt, tag="agout")
            nc.gpsimd.collective_compute(
                "AllGather", mybir.AluOpType.bypass, replica_groups=RG,
                ins=[agin[:]], outs=[agout[:]])
            # Gather the rank-interleaved AG output into contiguous [N, B]
            # DRAM buffers (one DMA each), then load SBUF state with one DMA
            # per tensor -- keeps the semaphore fan-in per consumer tiny.
            us_lin = dram.tile([N, B], dt, tag="uslin")
            uf_lin = dram.tile([N, B], dt, tag="uflin")
            agv = agout[:].rearrange("(r t q) b -> r t q b", t=2, q=LOC)
            nc.gpsimd.dma_start(
                us_lin[:].rearrange("(r q) b -> r q b", q=LOC), agv[:, 0])
            nc.gpsimd.dma_start(
                uf_lin[:].rearrange("(r q) b -> r q b", q=LOC), agv[:, 1])
            us_nx = statep.tile([128, KT, B], dt, tag="us")
            uf_nx = statep.tile([128, KT, B], dt, tag="uf")
            nc.gpsimd.dma_start(
                us_nx[:], us_lin[:].rearrange("(k p) b -> p k b", p=128))
            nc.gpsimd.dma_start(
                uf_nx[:], uf_lin[:].rearrange("(k p) b -> p k b", p=128))
            return agin, us_nx, uf_nx

        _, us_cur, uf_cur = exchange(news0, newf0)

        # ---- 10 Euler steps ----
        last_agin = None
        for t in range(nsteps):
            # rank-R projections p_sT = MsB.T @ ufT, p_fT = MfB.T @ usT
            pps = psum.tile([R, B], dt, tag="mm")
            for k in range(KT):
                nc.tensor.matmul(pps[:], MsB_sb[:, k, :], uf_cur[:, k, :],
                                 start=(k == 0), stop=(k == KT - 1))
            ppf = psum.tile([R, B], dt, tag="mm")
            for k in range(KT):
                nc.tensor.matmul(ppf[:], MfB_sb[:, k, :], us_cur[:, k, :],
                                 start=(k == 0), stop=(k == KT - 1))
            ps_sb = smallp.tile([R, B], dt, tag="p")
            nc.vector.tensor_copy(ps_sb[:], pps[:])
            pf_sb = smallp.tile([R, B], dt, tag="p")
            nc.vector.tensor_copy(pf_sb[:], ppf[:])

            news = newp.tile([128, MT, B], dt, tag="new")
            newf = newp.tile([128, MT, B], dt, tag="newf")
            for phase in range(2):
                A_d = As_d if phase == 0 else Af_d
                st = us_cur if phase == 0 else uf_cur
                MA = MsA_sb if phase == 0 else MfA_sb
                pp = ps_sb if phase == 0 else pf_sb
                dst = news if phase == 0 else newf
                mps = [psum.tile([128, B], dt, tag="mm", name=f"mm_{t}_{phase}_{m}")
                       for m in range(MT)]
                for k in range(KT):
                    slab = slabp.tile([128, LOC], dt, tag="slab")
                    nc.sync.dma_start(slab[:], A_d[k * 128:(k + 1) * 128, :])
                    for m in range(MT):
                        nc.tensor.matmul(mps[m][:], slab[:, m * 128:(m + 1) * 128],
                                         st[:, k, :], start=(k == 0), stop=False)
                for m in range(MT):
                    nc.tensor.matmul(mps[m][:], MA[:, m * 128:(m + 1) * 128], pp[:],
                                     start=False, stop=True)
                    nc.scalar.activation(dst[:, m, :], mps[m][:], AF.Relu)

            last_agin, us_cur, uf_cur = exchange(news, newf)

        # final local state -> outputs (DRAM->DRAM copy out of the AG input)
        nc.sync.dma_start(us_o[:], last_agin[0:LOC, :])
        nc.sync.dma_start(uf_o[:], last_agin[LOC:2 * LOC, :])

        # ---- decoder:  x1 = softplus(relu((us+uf)@D1+db1)@D2+db2) ----
        lat = statep.tile([128, KT, B], dt, tag="lat")
        nc.vector.tensor_add(lat[:], us_cur[:], uf_cur[:])
        hdp = psum.tile([H, B], dt, tag="mm")
        for k in range(KT):
            d1t = mlpp.tile([128, H], dt, tag="w1")
            nc.sync.dma_start(d1t[:], D1_d[k * 128:(k + 1) * 128, :])
            nc.tensor.matmul(hdp[:], d1t[:], lat[:, k, :],
                             start=(k == 0), stop=(k == KT - 1))
        hdT = smallp.tile([H, B], dt, tag="hT")
        nc.scalar.activation(hdT[:], hdp[:], AF.Relu, bias=db1_sb[:, 0:1])
        d2_sb = mlpp.tile([H, LOC], dt, tag="w2")
        nc.sync.dma_start(d2_sb[:], D2_d[:])
        x1sb = newp.tile([128, MT, B], dt, tag="new")
        for m in range(MT):
            xp = psum.tile([128, B], dt, tag="mm")
            nc.tensor.matmul(xp[:], d2_sb[:, m * 128:(m + 1) * 128], hdT[:],
                             start=True, stop=True)
            # stable softplus(x+db2) = relu(x+db2) + ln(1+exp(-|x+db2|))
            xa = smallp.tile([128, B], dt, tag="xa")
            nc.scalar.activation(xa[:], xp[:], AF.Abs, bias=db2_sb[:, m:m + 1])
            nc.scalar.activation(xa[:], xa[:], AF.Exp, scale=-1.0)
            nc.scalar.activation(xa[:], xa[:], AF.Ln, bias=1.0)
            xr = smallp.tile([128, B], dt, tag="xr")
            nc.scalar.activation(xr[:], xp[:], AF.Relu, bias=db2_sb[:, m:m + 1])
            nc.vector.tensor_add(x1sb[:, m, :], xr[:], xa[:])
        nc.sync.dma_start(x1_o[:].rearrange("(m p) b -> p m b", p=128), x1sb[:])

    nc.compile()
    return nc


def make_in_maps(inputs, N=N_FULL, B=B_FULL, H=H_FULL, R=R_FULL,
                 ncores=NCORES):
    """Host-side prep: fold scalars/identity into the streamed matrices and
    shard across cores.  Returns a list of per-core input dicts."""
    LOC = N // ncores
    MT = LOC // 128
    f32 = np.float32

    def softplus(x):
        return np.log1p(np.exp(np.float64(x)))

    a_s = f32(DT * (softplus(inputs["raw_cs"]) + 1e-4))
    a_f = f32(DT * (softplus(inputs["raw_cf"]) + 1e-4))
    b_s = f32(DT * (softplus(inputs["raw_lambda_s"]) + 1e-4))
    b_f = f32(DT * (softplus(inputs["raw_lambda_f"]) + 1e-4))

    Ls = np.asarray(inputs["Ls"], f32)
    Lf = np.asarray(inputs["Lf"], f32)
    x0 = np.asarray(inputs["x0"], f32)
    x0T = np.ascontiguousarray(x0.T)

    com = {
        "x0T": x0T,
        "MsB": np.ascontiguousarray(np.asarray(inputs["Ms_B"], f32)),
        "MfB": np.ascontiguousarray(np.asarray(inputs["Mf_B"], f32)),
        "W1": np.ascontiguousarray(np.asarray(inputs["W1"], f32)),
        "b1": np.ascontiguousarray(np.asarray(inputs["b1"], f32).reshape(H, 1)),
        "D1": np.ascontiguousarray(np.asarray(inputs["D1"], f32)),
        "db1": np.ascontiguousarray(np.asarray(inputs["db1"], f32).reshape(H, 1)),
    }
    W2 = np.asarray(inputs["W2"], f32)
    D2 = np.asarray(inputs["D2"], f32)
    b2 = np.asarray(inputs["b2"], f32)
    db2 = np.asarray(inputs["db2"], f32)
    MsA = np.asarray(inputs["Ms_A"], f32)
    MfA = np.asarray(inputs["Mf_A"], f32)

    in_maps = []
    diag = np.arange(LOC)
    for c in range(ncores):
        r0, r1 = c * LOC, (c + 1) * LOC
        As_c = (-a_s) * Ls[r0:r1, :].T
        As_c[r0 + diag, diag] += f32(1.0)
        Af_c = (-a_f) * Lf[r0:r1, :].T
        Af_c[r0 + diag, diag] += f32(1.0)
        m = dict(com)
        m.update({
            "x0Tl": np.ascontiguousarray(x0T[r0:r1]),
            "As": np.ascontiguousarray(As_c),
            "Af": np.ascontiguousarray(Af_c),
            "MsA": np.ascontiguousarray(b_s * MsA[r0:r1].T),
            "MfA": np.ascontiguousarray(b_f * MfA[r0:r1].T),
            "W2": np.ascontiguousarray(W2[:, r0:r1]),
            "b2T": np.ascontiguousarray(b2[r0:r1].reshape(MT, 128).T),
            "D2": np.ascontiguousarray(D2[:, r0:r1]),
            "db2T": np.ascontiguousarray(db2[r0:r1].reshape(MT, 128).T),
        })
        in_maps.append(m)
    return in_maps, (a_s, a_f, b_s, b_f)


_PROGRAM_CACHE = {}


def kernel(**inputs):
    """Full-input / full-output entry point for the harness."""
    in_maps, _scal = make_in_maps(inputs)
    key = "full"
    if key not in _PROGRAM_CACHE:
        _PROGRAM_CACHE[key] = build_program()
    nc = _PROGRAM_CACHE[key]

    res = run_bass_kernel_spmd(nc, in_maps, core_ids=list(range(NCORES)))

    x1T = np.concatenate([res.results[c]["x1_o"] for c in range(NCORES)], axis=0)
    usT = np.concatenate([res.results[c]["us_o"] for c in range(NCORES)], axis=0)
    ufT = np.concatenate([res.results[c]["uf_o"] for c in range(NCORES)], axis=0)
    x1 = np.ascontiguousarray(x1T.T).astype(np.float32)
    us = np.ascontiguousarray(usT.T).astype(np.float32)
    uf = np.ascontiguousarray(ufT.T).astype(np.float32)
    return (x1, us, uf)
